# revision 1
# baseline (speedup 1.0000x reference)
import sys

if "/opt/trn_rl_repo" not in sys.path:
    sys.path.insert(0, "/opt/trn_rl_repo")

from contextlib import ExitStack

import numpy as np

import concourse.bass as bass
import concourse.bacc as bacc
import concourse.tile as tile
import concourse.mybir as mybir
from concourse.bass_utils import run_bass_kernel_spmd
from concourse.masks import make_identity

F32 = mybir.dt.float32
Alu = mybir.AluOpType
Act = mybir.ActivationFunctionType

B, FCH, H, W = 2, 48, 512, 512
RS = 128           # stripe rows per core
SH = 4             # stripes per batch
NCORES = 8
HALO = 3           # rows r0-1, r0+128, r0+129
FD = W + 4 * HALO  # 524 = 512 main cols + packed halo cols
OC = 34
PI = float(np.pi)
EPS = 1e-7

TRACE = False
LAST_RESULTS = None
_NC = None


def _emit(nc, tc, ctx, xin, band_d, bandh_d, icnt2_d, y):
    V, G, S = nc.vector, nc.gpsimd, nc.scalar
    main = ctx.enter_context(tc.tile_pool(name="main", bufs=1))
    tmpv = ctx.enter_context(tc.tile_pool(name="tmpv", bufs=4))
    tmpg = ctx.enter_context(tc.tile_pool(name="tmpg", bufs=4))
    pbig = ctx.enter_context(tc.tile_pool(name="pbig", bufs=1, space="PSUM"))
    psm = ctx.enter_context(tc.tile_pool(name="psm", bufs=2, space="PSUM"))

    def P(nm, tag=None):
        return main.tile([RS, FD], F32, name=nm, tag=tag or nm)

    Ipl = [[P(f"i{r}{c}") for c in range(4)] for r in range(4)]
    A = [[P(f"a{r}{c}") for c in range(4)] for r in range(4)]
    WT = [[P(f"w{r}{c}") for c in range(4)] for r in range(4)]
    invA = [P(f"ja{k}") for k in range(4)]
    invW = [P(f"jw{k}") for k in range(4)]
    rm = P("rm")
    rcp = P("rcp", tag="rm")   # disjoint lifetimes; share the slot
    c0 = P("c0")
    s0 = P("s0")
    ident = main.tile([RS, RS], F32)
    band_sb = main.tile([RS, RS], F32)
    bandh_sb = main.tile([HALO, RS], F32)
    icnt2_sb = main.tile([RS, W], F32)
    at = main.tile([RS, W], F32)
    azi = main.tile([RS, W], F32)
    Sc = main.tile([RS, W], F32)
    Ss = main.tile([RS, W], F32)
    Rc = main.tile([RS, W], F32)
    Rs = main.tile([RS, W], F32)
    q2 = main.tile([RS, W], F32)
    Lq = main.tile([RS, W], F32)
    std = main.tile([RS, W], F32)
    tcs = [main.tile([HALO, RS], F32, name="tcs") for _ in range(8)]

    # ---------------- loads ----------------
    # A planes + window consts issued from the gpsimd queue (cheap triggers)
    for r in range(4):
        for c in range(4):
            G.dma_start(out=A[r][c], in_=xin[16 + 4 * r + c])
    G.dma_start(out=band_sb, in_=band_d[:, :])
    G.dma_start(out=bandh_sb, in_=bandh_d[:, :])
    G.dma_start(out=icnt2_sb, in_=icnt2_d[:, :])
    # WT[r][c] = Wm[c][r] = ch 32+4c+r ; load in (r,c) order so diag comes early
    for r in range(4):
        for c in range(4):
            S.dma_start(out=WT[r][c], in_=xin[32 + 4 * c + r])
    for r in range(4):
        for c in range(4):
            nc.sync.dma_start(out=Ipl[r][c], in_=xin[4 * r + c])

    # ---------------- helpers ----------------
    def mac(e, tp, dst, a, b):
        t = tp.tile([RS, FD], F32, name="mt")
        e.tensor_mul(t, a, b)
        e.tensor_sub(dst, dst, t)

    def recip(dst, src):
        t = tmpv.tile([RS, FD], F32, name="rs", bufs=2)
        V.reciprocal_approx_accurate(out=dst, in_=src, scratch=t)

    def lu_step(m, inv, k, e, tp):
        for i in range(k + 1, 4):
            e.tensor_mul(m[i][k], m[i][k], inv[k])
            for j in range(k + 1, 4):
                mac(e, tp, m[i][j], m[i][k], m[k][j])

    def fwd_col(m, rhs, c, e, tp):
        for i in range(1, 4):
            for k in range(i):
                mac(e, tp, rhs[i][c], m[i][k], rhs[k][c])

    def back_col(m, inv, rhs, c, e, tp):
        for i in range(3, -1, -1):
            for j in range(i + 1, 4):
                mac(e, tp, rhs[i][c], m[i][j], rhs[j][c])
            e.tensor_mul(rhs[i][c], rhs[i][c], inv[i])

    # stage-2 RHS view: rhs2[i][c] = Ipl[c][i]  (solve WT X = Z^T, X = M^T)
    M2 = [[Ipl[c][i] for c in range(4)] for i in range(4)]

    # ---- LU + solves in topological emission order (tile tracks deps in
    # emission order; per-engine queues overlap where deps allow) ----
    make_identity(nc, ident)
    recip(invW[0], WT[0][0])
    recip(invA[0], A[0][0])
    lu_step(WT, invW, 0, G, tmpg)
    lu_step(A, invA, 0, V, tmpv)
    recip(invA[1], A[1][1])
    lu_step(A, invA, 1, V, tmpv)
    recip(invW[1], WT[1][1])
    lu_step(WT, invW, 1, G, tmpg)
    recip(invA[2], A[2][2])
    lu_step(A, invA, 2, V, tmpv)
    recip(invW[2], WT[2][2])
    lu_step(WT, invW, 2, G, tmpg)
    recip(invA[3], A[3][3])
    recip(invW[3], WT[3][3])

    # stage 1: solve A Z = I (Z left in Ipl)
    fwd_col(A, Ipl, 3, G, tmpg)
    fwd_col(A, Ipl, 0, V, tmpv)
    back_col(A, invA, Ipl, 3, G, tmpg)
    fwd_col(A, Ipl, 1, V, tmpv)
    fwd_col(A, Ipl, 2, V, tmpv)
    back_col(A, invA, Ipl, 0, V, tmpv)
    back_col(A, invA, Ipl, 1, V, tmpv)
    back_col(A, invA, Ipl, 2, V, tmpv)

    # stage 2: solve WT X = Z^T in place (X = M^T -> M[r][c] in Ipl[r][c])
    fwd_col(WT, M2, 3, V, tmpv)
    fwd_col(WT, M2, 0, G, tmpg)
    back_col(WT, invW, M2, 3, V, tmpv)
    for c in range(4):
        nc.sync.dma_start(out=y[12 + c], in_=Ipl[3][c][:, 0:W])
    back_col(WT, invW, M2, 0, G, tmpg)
    for c in range(4):
        nc.sync.dma_start(out=y[c], in_=Ipl[0][c][:, 0:W])
    fwd_col(WT, M2, 1, G, tmpg)
    back_col(WT, invW, M2, 1, G, tmpg)
    for c in range(4):
        nc.sync.dma_start(out=y[4 + c], in_=Ipl[1][c][:, 0:W])

    # ---------------- azimuth: azi = mod(0.5*atan2(M31, -M32), pi) ----------
    # HW Arctan is only valid on [-pi/2, pi/2]; for |t|>1 use
    # atan(t) = sign(t)*pi/2 - atan(1/t) with both branch inputs clipped.
    m31, m32 = Ipl[3][1], Ipl[3][2]
    recip(rm, m32)
    tz = tmpv.tile([RS, FD], F32, bufs=1)
    py = tmpv.tile([RS, FD], F32, bufs=1)
    sn = tmpv.tile([RS, FD], F32, bufs=1)
    f = tmpv.tile([RS, FD], F32, bufs=1)
    V.tensor_mul(tz[:, 0:W], m31[:, 0:W], rm[:, 0:W])          # t
    V.tensor_scalar(out=f[:, 0:W], in0=tz[:, 0:W], scalar1=-1.0, scalar2=1.0,
                    op0=Alu.max, op1=Alu.min)
    S.activation(out=at, in_=f[:, 0:W], func=Act.Arctan)       # a1
    recip(rm, m31)
    V.tensor_mul(f[:, 0:W], m32[:, 0:W], rm[:, 0:W])           # 1/t (exact)
    V.tensor_scalar(out=f[:, 0:W], in0=f[:, 0:W], scalar1=-1.0, scalar2=1.0,
                    op0=Alu.max, op1=Alu.min)
    S.activation(out=azi, in_=f[:, 0:W], func=Act.Arctan)      # a2 (azi scratch)
    V.tensor_mul(sn[:, 0:W], tz[:, 0:W], tz[:, 0:W])
    V.tensor_scalar(out=sn[:, 0:W], in0=sn[:, 0:W], scalar1=1.0, scalar2=None,
                    op0=Alu.is_le)                             # m = [|t|<=1]
    V.tensor_scalar(out=f[:, 0:W], in0=tz[:, 0:W], scalar1=0.0, scalar2=PI,
                    op0=Alu.is_ge, op1=Alu.mult)               # pi*[t>=0]
    V.scalar_tensor_tensor(out=azi, in0=azi, scalar=-1.0, in1=f[:, 0:W],
                           op0=Alu.mult, op1=Alu.add)          # pi*[t>=0] - a2
    V.tensor_scalar(out=azi, in0=azi, scalar1=PI / 2, scalar2=None,
                    op0=Alu.subtract)                          # alt branch
    V.tensor_sub(at, at, azi)
    V.tensor_mul(at, at, sn[:, 0:W])
    V.tensor_add(at, at, azi)                                  # atan(t), full range
    # quadrant fixup: azi = -0.5*at + (py + [x>0]*(pi/2 - py)), py = pi*[y<0]
    V.tensor_scalar(out=py[:, 0:W], in0=m31[:, 0:W], scalar1=0.0, scalar2=PI,
                    op0=Alu.is_lt, op1=Alu.mult)
    V.tensor_scalar(out=sn[:, 0:W], in0=m32[:, 0:W], scalar1=0.0, scalar2=None,
                    op0=Alu.is_gt)
    V.tensor_scalar(out=f[:, 0:W], in0=py[:, 0:W], scalar1=-1.0, scalar2=PI / 2,
                    op0=Alu.mult, op1=Alu.add)
    V.tensor_mul(f[:, 0:W], f[:, 0:W], sn[:, 0:W])
    V.tensor_add(py[:, 0:W], py[:, 0:W], f[:, 0:W])
    V.scalar_tensor_tensor(out=azi, in0=at, scalar=-0.5, in1=py[:, 0:W],
                           op0=Alu.mult, op1=Alu.add)
    nc.sync.dma_start(out=y[16], in_=azi)

    # cos/sin of 2*azi over full width (incl. halo cols): x/r, y/r
    xx = tmpv.tile([RS, FD], F32, bufs=1)
    yy = tmpv.tile([RS, FD], F32, bufs=1)
    S.activation(out=xx, in_=m32, func=Act.Square)
    S.activation(out=yy, in_=m31, func=Act.Square)
    r2 = tmpv.tile([RS, FD], F32, bufs=1)
    V.tensor_add(r2, xx, yy)
    rr = tmpv.tile([RS, FD], F32, bufs=1)
    S.activation(out=rr, in_=r2, func=Act.Sqrt)
    recip(rcp, rr)
    V.tensor_mul(c0, m32, rcp)   # -cos(2azi); sign dies in the square
    V.tensor_mul(s0, m31, rcp)   # sin(2azi)

    # ---------------- DVE: stage-2 chain 2 ----------------
    fwd_col(WT, M2, 2, V, tmpv)
    back_col(WT, invW, M2, 2, V, tmpv)
    for c in range(4):
        nc.sync.dma_start(out=y[8 + c], in_=Ipl[2][c][:, 0:W])

    # ---------------- window sums: rows on PE, cols on DVE ----------------
    for j in range(4):
        pt = psm.tile([HALO, RS], F32, name="pt")
        nc.tensor.transpose(pt, c0[:, W + 3 * j: W + 3 * j + 3], ident)
        S.activation(out=tcs[j], in_=pt, func=Act.Copy)
    for j in range(4):
        pt = psm.tile([HALO, RS], F32, name="pt")
        nc.tensor.transpose(pt, s0[:, W + 3 * j: W + 3 * j + 3], ident)
        S.activation(out=tcs[4 + j], in_=pt, func=Act.Copy)

    Pc = pbig.tile([RS, W], F32)
    Ps = pbig.tile([RS, W], F32)
    nc.tensor.matmul(Pc, band_sb, c0[:, 0:W], start=True, stop=True)
    nc.tensor.matmul(Ps, band_sb, s0[:, 0:W], start=True, stop=True)

    # row sums: Rc/Rs = band part + halo part (each halo matmul its own group)
    S.activation(out=Rc, in_=Pc, func=Act.Copy)
    S.activation(out=Rs, in_=Ps, func=Act.Copy)
    for j in range(4):
        ph = psm.tile([RS, RS], F32, name="ph")
        nc.tensor.matmul(ph, bandh_sb, tcs[j], start=True, stop=True)
        V.tensor_add(Rc[:, RS * j: RS * (j + 1)],
                     Rc[:, RS * j: RS * (j + 1)], ph)
    for j in range(4):
        ph = psm.tile([RS, RS], F32, name="ph")
        nc.tensor.matmul(ph, bandh_sb, tcs[4 + j], start=True, stop=True)
        V.tensor_add(Rs[:, RS * j: RS * (j + 1)],
                     Rs[:, RS * j: RS * (j + 1)], ph)

    S.activation(out=Sc, in_=Rc, func=Act.Copy)
    S.activation(out=Ss, in_=Rs, func=Act.Copy)
    V.tensor_add(Sc[:, 0:W - 1], Sc[:, 0:W - 1], Rc[:, 1:W])
    V.tensor_add(Sc[:, 0:W - 2], Sc[:, 0:W - 2], Rc[:, 2:W])
    V.tensor_add(Sc[:, 1:W], Sc[:, 1:W], Rc[:, 0:W - 1])
    V.tensor_add(Ss[:, 0:W - 1], Ss[:, 0:W - 1], Rs[:, 1:W])
    V.tensor_add(Ss[:, 0:W - 2], Ss[:, 0:W - 2], Rs[:, 2:W])
    V.tensor_add(Ss[:, 1:W], Ss[:, 1:W], Rs[:, 0:W - 1])

    xq = tmpv.tile([RS, FD], F32, tag="xx", bufs=1)
    yq = tmpv.tile([RS, FD], F32, tag="yy", bufs=1)
    S.activation(out=xq[:, 0:W], in_=Sc, func=Act.Square)
    S.activation(out=yq[:, 0:W], in_=Ss, func=Act.Square)
    V.tensor_add(q2, xq[:, 0:W], yq[:, 0:W])
    V.tensor_mul(q2, q2, icnt2_sb)
    V.tensor_scalar(out=q2, in0=q2, scalar1=float(EPS * EPS),
                    scalar2=float((1.0 - EPS) ** 2), op0=Alu.max, op1=Alu.min)
    S.activation(out=Lq, in_=q2, func=Act.Ln)
    S.activation(out=std, in_=Lq, func=Act.Sqrt, scale=-0.25)
    nc.sync.dma_start(out=y[17], in_=std)


def _build():
    nc = bacc.Bacc(target_bir_lowering=False)
    xin = nc.dram_tensor("xin", [FCH, RS, FD], F32, kind="ExternalInput")
    band_d = nc.dram_tensor("band", [RS, RS], F32, kind="ExternalInput")
    bandh_d = nc.dram_tensor("bandh", [HALO, RS], F32, kind="ExternalInput")
    icnt2_d = nc.dram_tensor("icnt2", [RS, W], F32, kind="ExternalInput")
    y = nc.dram_tensor("y", [OC - 16, RS, W], F32, kind="ExternalOutput")
    with tile.TileContext(nc) as tc, ExitStack() as ctx:
        _emit(nc, tc, ctx, xin, band_d, bandh_d, icnt2_d, y)
    nc.finalize()
    return nc


def _consts(core):
    b_, s_ = divmod(core, SH)
    r0 = s_ * RS
    g = np.arange(RS)[:, None]
    i = np.arange(RS)[None, :]
    band = ((g >= i - 1) & (g <= i + 2)).astype(np.float32)
    bandh = np.zeros((HALO, RS), np.float32)
    if r0 > 0:
        bandh[0, 0] = 1.0
    if r0 + RS <= H - 1:
        bandh[1, 126] = 1.0
        bandh[1, 127] = 1.0
    if r0 + RS + 1 <= H - 1:
        bandh[2, 127] = 1.0
    gi = r0 + np.arange(RS)
    rowc = np.minimum(gi + 2, H - 1) - np.maximum(gi - 1, 0) + 1
    w = np.arange(W)
    colc = np.minimum(w + 2, W - 1) - np.maximum(w - 1, 0) + 1
    cnt = (rowc[:, None] * colc[None, :]).astype(np.float64)
    icnt2 = (1.0 / (cnt * cnt)).astype(np.float32)
    return band, bandh, icnt2


def _pack(x, core):
    b_, s_ = divmod(core, SH)
    r0 = s_ * RS
    xb = x[b_]
    mainp = xb[:, r0:r0 + RS, :]
    hrows = np.clip(np.array([r0 - 1, r0 + RS, r0 + RS + 1]), 0, H - 1)
    halo = xb[:, hrows, :]                       # (48, 3, 512), finite values
    ph = halo.reshape(FCH, HALO, SH, RS).transpose(0, 3, 2, 1) \
             .reshape(FCH, RS, 4 * HALO)         # packed col = 3j + h
    return np.ascontiguousarray(
        np.concatenate([mainp, ph], axis=2), dtype=np.float32)


class _Runner:
    """Build the jitted shard_map program ONCE; run_bass_kernel_spmd would
    re-trace + re-lower (re-serializing nc into the HLO) on every call."""

    def __init__(self, nc):
        import jax
        from concourse import bass2jax

        bass2jax.install_neuronx_cc_hook()
        pname = (nc.partition_id_tensor.name
                 if nc.partition_id_tensor is not None else None)
        in_names, out_names, out_avals, zeros = [], [], [], []
        for alloc in nc.m.functions[0].allocations:
            if not isinstance(alloc, mybir.MemoryLocationSet):
                continue
            name = alloc.memorylocations[0].name
            if alloc.kind == "ExternalInput":
                if name != pname:
                    in_names.append(name)
            elif alloc.kind == "ExternalOutput":
                shape = tuple(alloc.tensor_shape)
                dtype = mybir.dt.np(alloc.dtype)
                out_names.append(name)
                out_avals.append(jax.core.ShapedArray(shape, dtype))
                zeros.append(np.zeros((NCORES * shape[0], *shape[1:]), dtype))
        n_params = len(in_names)
        all_names = tuple(in_names + out_names
                          + ([pname] if pname is not None else []))

        def _body(*args):
            operands = list(args)
            if pname is not None:
                operands.append(bass2jax.partition_id_tensor())
            return tuple(bass2jax._bass_exec_p.bind(
                *operands,
                out_avals=tuple(out_avals),
                in_names=all_names,
                out_names=tuple(out_names),
                lowering_input_output_aliases=(),
                sim_require_finite=True,
                sim_require_nnan=True,
                nc=nc,
            ))

        devices = jax.devices()[:NCORES]
        mesh = bass2jax.Mesh(np.asarray(devices), ("core",))
        spec = bass2jax.PartitionSpec("core")
        self.fn = jax.jit(
            bass2jax.shard_map(
                _body, mesh=mesh,
                in_specs=(spec,) * (n_params + len(out_names)),
                out_specs=(spec,) * len(out_names),
                check_rep=False),
            donate_argnums=tuple(range(n_params, n_params + len(out_names))),
            keep_unused=True)
        self.in_names = in_names
        self.out_avals = out_avals
        self.zeros = zeros

    def __call__(self, concat_ins):
        outs = self.fn(*concat_ins, *self.zeros)
        return [np.asarray(o) for o in outs]


_RUNNER = None
_CONST_INS = None


def kernel(x):
    global _NC, _RUNNER, _CONST_INS
    x = np.asarray(x, dtype=np.float32)
    if _NC is None:
        _NC = _build()
        _RUNNER = _Runner(_NC)
        cb = [np.empty((NCORES * RS, RS), np.float32),
              np.empty((NCORES * HALO, RS), np.float32),
              np.empty((NCORES * RS, W), np.float32)]
        for core in range(NCORES):
            band, bandh, icnt2 = _consts(core)
            cb[0][core * RS:(core + 1) * RS] = band
            cb[1][core * HALO:(core + 1) * HALO] = bandh
            cb[2][core * RS:(core + 1) * RS] = icnt2
        _CONST_INS = cb
    xcat = np.empty((NCORES * FCH, RS, FD), np.float32)
    for core in range(NCORES):
        xcat[core * FCH:(core + 1) * FCH] = _pack(x, core)
    ins = {"xin": xcat, "band": _CONST_INS[0],
           "bandh": _CONST_INS[1], "icnt2": _CONST_INS[2]}
    y = _RUNNER([ins[n] for n in _RUNNER.in_names])[0]
    y = y.reshape(NCORES, OC - 16, RS, W)
    out = np.empty((B, OC, H, W), np.float32)
    out[:, :16] = x[:, :16]
    for core in range(NCORES):
        b_, s_ = divmod(core, SH)
        out[b_, 16:, s_ * RS:(s_ + 1) * RS, :] = y[core]
    return out

